# revision 9
# baseline (speedup 1.0000x reference)
"""Distributed Trainium2 kernel for the GNN message-passing model.

Self-contained: host-side structural prep (sharding, edge sort, index
remap) + Bass/Tile SPMD kernel across 8 NeuronCores.

Math (see reference):
  logits = MLP(x1); m = 0.15 + 0.55*onehot(argmax(logits))
  r1 = (m@W1.sum(-1))*x2 + m@bp1
  g1 = relu(Dh A Dh (r1@gcn1_w) + gcn1_b); g1 = (m@W12)*g1 + 2e-4*(r1@W13)
  r2 = (m@W2.sum(-1))*g1 + m@bp2
  g2 = relu(Dh A Dh (r2@gcn2_w) + gcn2_b)
  out = log_softmax(g2@fc_w + fc_b)
where Dh = diag(deg^-1/2), deg = in-degree over dst.

Distribution: nodes sharded contiguously over 8 cores. Per GCN layer the
scaled features h' = Dh*h are AllGathered (in node chunks, so comm
overlaps the producer pipeline); each core gathers h'[src] for edges
whose dst it owns via indirect DMA and scatter-reduces them with
one-hot matmuls on the TensorEngine (PSUM accumulation per dst block).
"""

import numpy as np

P = 128
TAU_HI = 0.7
TAU_LO = 0.15  # (1-0.7)/2


class _Cfg:
    def __init__(self, N, E, F1=768, H=512, G1=256, G2=32, FOUT=40, C=7):
        self.NC = 8
        self.N = N
        self.E = E
        self.NLOC_RAW = N // self.NC
        self.NB = -(-self.NLOC_RAW // P)          # node blocks per core
        self.NLOC = self.NB * P
        assert self.NB % C == 0, (self.NB, C)
        self.C = C                                 # allgather chunks
        self.BPC = self.NB // C                    # blocks per chunk
        self.CH = self.BPC * P                     # chunk nodes
        self.TR = self.NC * self.NLOC              # gathered table rows
        self.CHR = self.NC * self.CH               # rows per chunk in table
        self.F1, self.H, self.G1, self.G2, self.FOUT = F1, H, G1, G2, FOUT
        self.KF1 = F1 // P                         # 6 k-tiles
        self.KH = H // P                           # 4
        self.KG1 = G1 // P                         # 2
        self.NFREE = min(448, self.CH)             # front free-dim unit
        assert self.CH % self.NFREE == 0
        self.FU = self.CH // self.NFREE            # free units per chunk


CFG_FULL = dict(N=50000, E=800000)


def _to_bf16(x):
    import ml_dtypes
    return np.asarray(x, np.float32).astype(ml_dtypes.bfloat16)


def _row_of_node(v, cfg):
    """Gathered-table row for global node id v (vectorized)."""
    c = v // cfg.NLOC_RAW
    s = v - c * cfg.NLOC_RAW
    k = s // cfg.CH
    return k * cfg.CHR + c * cfg.CH + (s - k * cfg.CH)


def host_prep(inputs, cfg):
    """Returns (in_maps, sched). sched is baked into the built graph and
    must be identical for every core (SPMD)."""
    x1 = np.asarray(inputs["x1"], np.float32)
    x2 = np.asarray(inputs["x2"], np.float32)
    ei = np.asarray(inputs["edge_index"])
    src = ei[0].astype(np.int64)
    dst = ei[1].astype(np.int64)
    N, E, NC = cfg.N, cfg.E, cfg.NC
    assert x1.shape[0] == N and src.shape[0] == E

    deg = np.bincount(dst, minlength=N).astype(np.float64)
    dinv = np.where(deg > 0, deg ** -0.5, 0.0).astype(np.float32)
    sdeg = np.sqrt(deg).astype(np.float32)  # 1/dinv where deg>0 else 0

    # ---- per-core edge partition by dst owner, sorted by dst block ----
    owner = dst // cfg.NLOC_RAW
    dloc = dst - owner * cfg.NLOC_RAW
    dblk = dloc // P
    drel_all = (dloc - dblk * P).astype(np.float32)
    rows_all = _row_of_node(src, cfg).astype(np.int32)

    per_core = []
    cnt = np.zeros((NC, cfg.NB), np.int64)
    for c in range(NC):
        sel = np.where(owner == c)[0]
        order = np.argsort(dblk[sel], kind="stable")
        sel = sel[order]
        b_of = dblk[sel]
        bounds = np.searchsorted(b_of, np.arange(cfg.NB + 1))
        lists = []
        for b in range(cfg.NB):
            idxs = sel[bounds[b]:bounds[b + 1]]
            lists.append((rows_all[idxs], drel_all[idxs]))
            cnt[c, b] = len(idxs)
        per_core.append(lists)

    # uniform cross-core schedule
    Kb = np.maximum(1, -(-cnt.max(axis=0) // P)).astype(np.int64)
    total = int(Kb.sum())
    pad_to = 32
    if total % pad_to:
        Kb[cfg.NB - 1] += pad_to - (total % pad_to)
        total = int(Kb.sum())
    nblocks = total
    SB = nblocks // 8
    b_of_block = np.repeat(np.arange(cfg.NB), Kb)
    first_of_b = np.zeros(nblocks, bool)
    last_of_b = np.zeros(nblocks, bool)
    off = 0
    for b in range(cfg.NB):
        first_of_b[off] = True
        last_of_b[off + int(Kb[b]) - 1] = True
        off += int(Kb[b])

    # row of (core 0, slot NLOC-1): a padded slot whose h' row is 0
    dummy_row = np.int32((cfg.C - 1) * cfg.CHR +
                         (cfg.NLOC - 1 - (cfg.C - 1) * cfg.CH))

    sched = dict(Kb=Kb, nblocks=nblocks, SB=SB, b_of_block=b_of_block,
                 first_of_b=first_of_b, last_of_b=last_of_b)

    # ---- weights ----
    w1 = np.asarray(inputs["mlp_w1"], np.float32)
    w2 = np.asarray(inputs["mlp_w2"], np.float32)
    w3 = np.asarray(inputs["mlp_w3"], np.float32)
    b1 = np.asarray(inputs["mlp_b1"], np.float32)
    b2 = np.asarray(inputs["mlp_b2"], np.float32)
    b3 = np.asarray(inputs["mlp_b3"], np.float32)
    W1s = np.asarray(inputs["W1"], np.float32).sum(-1)
    W12 = np.asarray(inputs["W12"], np.float32)
    W13 = np.asarray(inputs["W13"], np.float32) * 2e-4
    bp1 = np.asarray(inputs["bp1"], np.float32)
    W2s = np.asarray(inputs["W2"], np.float32).sum(-1)
    bp2 = np.asarray(inputs["bp2"], np.float32)
    g1w = np.asarray(inputs["gcn1_w"], np.float32)
    g1b = np.asarray(inputs["gcn1_b"], np.float32)
    g2w = np.asarray(inputs["gcn2_w"], np.float32)
    g2b = np.asarray(inputs["gcn2_b"], np.float32)
    fcw = np.asarray(inputs["fc_w"], np.float32)
    fcb = np.asarray(inputs["fc_b"], np.float32)

    sched["bp1_nz"] = bool(np.any(bp1 != 0))
    sched["bp2_nz"] = bool(np.any(bp2 != 0))
    sched["g1b_nz"] = bool(np.any(g1b != 0))
    sched["g2b_nz"] = bool(np.any(g2b != 0))
    sched["fcb_nz"] = bool(np.any(fcb != 0))
    sched["b3_nz"] = bool(np.any(b3 != 0))

    def pack_lhsT(w, KT, MT):
        o = np.zeros((P, KT * MT * P), np.float32)
        for k in range(KT):
            for m in range(MT):
                o[:, (k * MT + m) * P:(k * MT + m + 1) * P] = \
                    w[k * P:(k + 1) * P, m * P:(m + 1) * P]
        return _to_bf16(o)

    def pack_rhs(w, KT, F):
        o = np.zeros((P, KT * F), np.float32)
        for k in range(KT):
            o[:, k * F:(k + 1) * F] = w[k * P:(k + 1) * P, :]
        return _to_bf16(o)

    def pack_k3(w, F):
        o = np.zeros((4, F), np.float32)
        o[:3] = w
        return _to_bf16(o)

    w1_p = pack_lhsT(w1, cfg.KF1, cfg.KH)
    w2_p = pack_lhsT(w2, cfg.KH, cfg.KH)
    w3_p = pack_rhs(np.pad(w3, ((0, 0), (0, 1))), cfg.KH, 4)
    b1_p = b1.reshape(cfg.KH, P).T.copy()
    b2_p = b2.reshape(cfg.KH, P).T.copy()
    b3_p = np.pad(b3, (0, 1)).reshape(1, 4).repeat(P, 0).copy()
    g1w_p = pack_rhs(g1w, cfg.KF1, cfg.G1)
    w13_p = pack_rhs(W13, cfg.KF1, cfg.G1)
    g2w_p = pack_rhs(g2w, cfg.KG1, cfg.G2)
    fcw_p = _to_bf16(fcw)
    W1s_p = pack_k3(W1s, cfg.F1)
    bp1_p = pack_k3(bp1, cfg.F1)
    W12_p = pack_k3(W12, cfg.G1)
    W2s_p = pack_k3(W2s, cfg.G1)
    bp2_p = pack_k3(bp2, cfg.G1)
    g1b_p = _to_bf16(g1b.reshape(1, cfg.G1))
    g2b_p = _to_bf16(g2b.reshape(1, cfg.G2))
    fcb_p = _to_bf16(fcb.reshape(1, cfg.FOUT))

    in_maps = []
    for c in range(NC):
        lo = c * cfg.NLOC_RAW
        hi = lo + cfg.NLOC_RAW
        x1T = np.zeros((cfg.F1, cfg.NLOC), np.float32)
        x1T[:, :cfg.NLOC_RAW] = x1[lo:hi].T
        x2T = np.zeros((cfg.F1, cfg.NLOC), np.float32)
        x2T[:, :cfg.NLOC_RAW] = x2[lo:hi].T
        dinv_t = np.zeros((P, cfg.NB), np.float32)
        dinv_t.T.reshape(-1)[:cfg.NLOC_RAW] = dinv[lo:hi]
        sdeg_r = np.zeros((1, cfg.NLOC), np.float32)
        sdeg_r[0, :cfg.NLOC_RAW] = sdeg[lo:hi]

        idx = np.full((SB * P, 8), dummy_row, np.int32)
        drl = np.full((SB * P, 8), -1.0, np.float32)
        off = 0
        for b in range(cfg.NB):
            rows, rel = per_core[c][b]
            n = len(rows)
            for j in range(int(Kb[b])):
                g = off + j
                s, jj = g // 8, g % 8
                e0 = j * P
                m = min(P, max(0, n - e0))
                if m > 0:
                    idx[s * P:s * P + m, jj] = rows[e0:e0 + m]
                    drl[s * P:s * P + m, jj] = rel[e0:e0 + m]
            off += int(Kb[b])

        ident_np = _to_bf16(np.eye(P, dtype=np.float32))
        iota_np = _to_bf16(np.tile(np.arange(P, dtype=np.float32), (P, 1)))
        im = {
            "ident": ident_np, "iota": iota_np,
            "x1T": _to_bf16(x1T), "x2T": _to_bf16(x2T),
            "idx": idx, "drl": _to_bf16(drl),
            "dinv_t": dinv_t, "sdeg_r": _to_bf16(sdeg_r),
            "w1": w1_p, "w2": w2_p, "w3": w3_p,
            "b1": b1_p, "b2": b2_p, "b3": b3_p,
            "g1w": g1w_p, "w13": w13_p, "g2w": g2w_p, "fcw": fcw_p,
            "W1s": W1s_p, "bp1": bp1_p, "W12": W12_p, "W2s": W2s_p,
            "bp2": bp2_p, "g1b": g1b_p, "g2b": g2b_p, "fcb": fcb_p,
        }
        in_maps.append(im)
    return in_maps, sched


def build(cfg, sched, debug=False):
    import concourse.bacc as bacc
    import concourse.bass as bass
    import concourse.mybir as mybir
    import concourse.tile as tile

    dt = mybir.dt
    AF = mybir.ActivationFunctionType
    OP = mybir.AluOpType
    AX = mybir.AxisListType

    nc = bacc.Bacc("TRN2", target_bir_lowering=False, debug=debug)

    NB, C, BPC, CH, NLOC, TR, CHR = (cfg.NB, cfg.C, cfg.BPC, cfg.CH,
                                     cfg.NLOC, cfg.TR, cfg.CHR)
    F1, H, G1, G2, FOUT = cfg.F1, cfg.H, cfg.G1, cfg.G2, cfg.FOUT
    KF1, KH, KG1 = cfg.KF1, cfg.KH, cfg.KG1
    NF, FU = cfg.NFREE, cfg.FU
    SB, nblocks = sched["SB"], sched["nblocks"]
    b_of_block = sched["b_of_block"]
    first_of_b, last_of_b = sched["first_of_b"], sched["last_of_b"]

    bf = dt.bfloat16
    f32 = dt.float32

    dd = {}

    def din(name, shape, dtype):
        dd[name] = nc.declare_dram_parameter(name, list(shape), dtype,
                                             isOutput=False)
        return dd[name]

    x1T_d = din("x1T", [F1, NLOC], bf)
    x2T_d = din("x2T", [F1, NLOC], bf)
    idx_d = din("idx", [SB * P, 8], dt.int32)
    drl_d = din("drl", [SB * P, 8], bf)
    dinv_d = din("dinv_t", [P, NB], f32)
    sdeg_d = din("sdeg_r", [1, NLOC], bf)
    w1_d = din("w1", [P, KF1 * KH * P], bf)
    w2_d = din("w2", [P, KH * KH * P], bf)
    w3_d = din("w3", [P, KH * 4], bf)
    b1_d = din("b1", [P, KH], f32)
    b2_d = din("b2", [P, KH], f32)
    b3_d = din("b3", [P, 4], f32)
    g1w_d = din("g1w", [P, KF1 * G1], bf)
    w13_d = din("w13", [P, KF1 * G1], bf)
    g2w_d = din("g2w", [P, KG1 * G2], bf)
    fcw_d = din("fcw", [G2, FOUT], bf)
    W1s_d = din("W1s", [4, F1], bf)
    bp1_d = din("bp1", [4, F1], bf)
    W12_d = din("W12", [4, G1], bf)
    W2s_d = din("W2s", [4, G1], bf)
    bp2_d = din("bp2", [4, G1], bf)
    g1b_d = din("g1b", [1, G1], bf)
    g2b_d = din("g2b", [1, G2], bf)
    fcb_d = din("fcb", [1, FOUT], bf)
    ident_d = din("ident", [P, P], bf)
    iota_d = din("iota", [P, P], bf)
    out_d = nc.declare_dram_parameter("out", [NLOC, FOUT], f32, isOutput=True)

    with tile.TileContext(nc) as tc:
        with (
            tc.tile_pool(name="const", bufs=1) as cp,
            tc.tile_pool(name="front", bufs=2) as fp,
            tc.tile_pool(name="scat", bufs=3) as sp,
            tc.tile_pool(name="fin", bufs=2) as qp,
            tc.tile_pool(name="psA", bufs=2, space="PSUM") as psA,
            tc.tile_pool(name="psB", bufs=2, space="PSUM") as psB,
            tc.tile_pool(name="psS", bufs=2, space="PSUM") as psS,
            tc.tile_pool(name="psT", bufs=2, space="PSUM") as psT,
            tc.tile_pool(name="dram", bufs=1, space="DRAM") as dp,
        ):
            def load(dr, shape, dtype, name):
                t = cp.tile(shape, dtype, tag=name)
                nc.sync.dma_start(out=t[:, :], in_=dr[:, :])
                return t

            w1_s = load(w1_d, [P, KF1 * KH * P], bf, "w1")
            w2_s = load(w2_d, [P, KH * KH * P], bf, "w2")
            w3_s = load(w3_d, [P, KH * 4], bf, "w3")
            b1_s = load(b1_d, [P, KH], f32, "b1")
            b2_s = load(b2_d, [P, KH], f32, "b2")
            b3_s = load(b3_d, [P, 4], f32, "b3")
            g1w_s = load(g1w_d, [P, KF1 * G1], bf, "g1w")
            w13_s = load(w13_d, [P, KF1 * G1], bf, "w13")
            g2w_s = load(g2w_d, [P, KG1 * G2], bf, "g2w")
            fcw_s = load(fcw_d, [G2, FOUT], bf, "fcw")
            W1s_s = load(W1s_d, [4, F1], bf, "W1s")
            bp1_s = load(bp1_d, [4, F1], bf, "bp1")
            W12_s = load(W12_d, [4, G1], bf, "W12")
            W2s_s = load(W2s_d, [4, G1], bf, "W2s")
            bp2_s = load(bp2_d, [4, G1], bf, "bp2")
            g1b_s = load(g1b_d, [1, G1], bf, "g1b")
            g2b_s = load(g2b_d, [1, G2], bf, "g2b")
            fcb_s = load(fcb_d, [1, FOUT], bf, "fcb")
            dinv_s = load(dinv_d, [P, NB], f32, "dinv")
            sdeg_s = load(sdeg_d, [1, NLOC], bf, "sdeg")

            ident = load(ident_d, [P, P], bf, "ident")
            iotaP = load(iota_d, [P, P], bf, "iotaP")
            ones1 = cp.tile([1, P], bf, tag="ones1")
            nc.vector.memset(ones1[:, :], 1.0)

            mT_s = cp.tile([4, NLOC], bf, tag="mT")
            z_s = cp.tile([P, NB * G1], bf, tag="z")

            h1bs = [dp.tile([CH, G1], bf, tag=f"h1b{k}", name=f"h1b{k}")
                    for k in range(C)]
            h2bs = [dp.tile([CH, G2], bf, tag=f"h2b{k}", name=f"h2b{k}")
                    for k in range(C)]
            h1g = dp.tile([TR, G1], bf, tag="h1g")
            h2g = dp.tile([TR, G2], bf, tag="h2g")

            # ================= FRONT (per chunk) =================
            for k in range(C):
                n0 = k * CH
                x1c = fp.tile([P, KF1 * CH], bf, tag="x1c")
                nc.sync.dma_start(
                    out=x1c[:, :].rearrange("p (a n) -> p a n", n=CH),
                    in_=x1T_d[:, n0:n0 + CH].rearrange("(a p) n -> p a n", p=P))
                x2c = fp.tile([P, KF1 * CH], bf, tag="x2c")
                nc.sync.dma_start(
                    out=x2c[:, :].rearrange("p (a n) -> p a n", n=CH),
                    in_=x2T_d[:, n0:n0 + CH].rearrange("(a p) n -> p a n", p=P))

                h1T = fp.tile([P, KH * CH], bf, tag="h1T")
                for u in range(FU):
                    for m in range(KH):
                        ps = psA.tile([P, NF], f32, tag="a")
                        for kk in range(KF1):
                            nc.tensor.matmul(
                                ps[:, :],
                                lhsT=w1_s[:, (kk * KH + m) * P:(kk * KH + m + 1) * P],
                                rhs=x1c[:, kk * CH + u * NF:kk * CH + u * NF + NF],
                                start=(kk == 0), stop=(kk == KF1 - 1))
                        nc.scalar.activation(
                            h1T[:, m * CH + u * NF:m * CH + u * NF + NF],
                            ps[:, :], AF.Relu, bias=b1_s[:, m:m + 1])
                h2T = fp.tile([P, KH * CH], bf, tag="h2T")
                for u in range(FU):
                    for m in range(KH):
                        ps = psA.tile([P, NF], f32, tag="a")
                        for kk in range(KH):
                            nc.tensor.matmul(
                                ps[:, :],
                                lhsT=w2_s[:, (kk * KH + m) * P:(kk * KH + m + 1) * P],
                                rhs=h1T[:, kk * CH + u * NF:kk * CH + u * NF + NF],
                                start=(kk == 0), stop=(kk == KH - 1))
                        nc.scalar.activation(
                            h2T[:, m * CH + u * NF:m * CH + u * NF + NF],
                            ps[:, :], AF.Relu, bias=b2_s[:, m:m + 1])

                for nb in range(BPC):
                    b_glob = k * BPC + nb
                    psl = psB.tile([P, G1], f32, tag="b")
                    for kk in range(KH):
                        nc.tensor.matmul(
                            psl[:, :4],
                            lhsT=h2T[:, kk * CH + nb * P:kk * CH + (nb + 1) * P],
                            rhs=w3_s[:, kk * 4:(kk + 1) * 4],
                            start=(kk == 0), stop=(kk == KH - 1))
                    lg = fp.tile([P, 3], f32, tag="lg")
                    if sched["b3_nz"]:
                        nc.vector.tensor_add(lg[:, :], psl[:, :3], b3_s[:, :3])
                    else:
                        nc.vector.tensor_copy(lg[:, :], psl[:, :3])
                    rmax = fp.tile([P, 1], f32, tag="rmax")
                    nc.vector.reduce_max(rmax[:, :], lg[:, :], axis=AX.X)
                    mm = fp.tile([P, 3], bf, tag="mm")
                    nc.vector.tensor_scalar(
                        mm[:, :], lg[:, :], rmax[:, :1], None, OP.is_equal)
                    mmf = fp.tile([P, 3], bf, tag="mmf")
                    nc.scalar.activation(mmf[:, :], mm[:, :], AF.Copy,
                                         bias=TAU_LO, scale=TAU_HI - TAU_LO)
                    pst = psT.tile([P, P], bf, tag="t")
                    nc.tensor.transpose(pst[:3, :], mmf[:, :3], ident[:, :])
                    nc.vector.tensor_copy(
                        mT_s[:3, b_glob * P:(b_glob + 1) * P], pst[:3, :])

                r1T = fp.tile([P, KF1 * CH], bf, tag="r1T")
                for u in range(FU):
                    for f in range(KF1):
                        psr = psA.tile([P, NF], f32, tag="a")
                        nc.tensor.matmul(
                            psr[:, :], lhsT=W1s_s[:3, f * P:(f + 1) * P],
                            rhs=mT_s[:3, n0 + u * NF:n0 + u * NF + NF],
                            start=True, stop=True)
                        if sched["bp1_nz"]:
                            psr2 = psB.tile([P, G1], f32, tag="b")
                            nc.tensor.matmul(
                                psr2[:, :NF], lhsT=bp1_s[:3, f * P:(f + 1) * P],
                                rhs=mT_s[:3, n0 + u * NF:n0 + u * NF + NF],
                                start=True, stop=True)
                            tmp = fp.tile([P, NF], f32, tag="r1tmp")
                            nc.vector.tensor_mul(
                                tmp[:, :], psr[:, :],
                                x2c[:, f * CH + u * NF:f * CH + u * NF + NF])
                            nc.vector.tensor_add(
                                r1T[:, f * CH + u * NF:f * CH + u * NF + NF],
                                tmp[:, :], psr2[:, :NF])
                        else:
                            nc.vector.tensor_mul(
                                r1T[:, f * CH + u * NF:f * CH + u * NF + NF],
                                psr[:, :],
                                x2c[:, f * CH + u * NF:f * CH + u * NF + NF])

                for nb in range(BPC):
                    b_glob = k * BPC + nb
                    psh = psA.tile([P, G1], f32, tag="a")
                    for f in range(KF1):
                        nc.tensor.matmul(
                            psh[:, :G1],
                            lhsT=r1T[:, f * CH + nb * P:f * CH + (nb + 1) * P],
                            rhs=g1w_s[:, f * G1:(f + 1) * G1],
                            start=(f == 0), stop=(f == KF1 - 1))
                    h1p = fp.tile([P, G1], bf, tag="h1p")
                    nc.scalar.activation(h1p[:, :], psh[:, :G1], AF.Copy,
                                         scale=dinv_s[:, b_glob:b_glob + 1])
                    nc.sync.dma_start(
                        out=h1bs[k][nb * P:(nb + 1) * P, :], in_=h1p[:, :])
                    psz = psA.tile([P, G1], f32, tag="a")
                    for f in range(KF1):
                        nc.tensor.matmul(
                            psz[:, :G1],
                            lhsT=r1T[:, f * CH + nb * P:f * CH + (nb + 1) * P],
                            rhs=w13_s[:, f * G1:(f + 1) * G1],
                            start=(f == 0), stop=(f == KF1 - 1))
                    nc.scalar.activation(
                        z_s[:, b_glob * G1:(b_glob + 1) * G1],
                        psz[:, :G1], AF.Copy)

                nc.gpsimd.collective_compute(
                    "AllGather", OP.bypass,
                    replica_groups=[list(range(cfg.NC))],
                    ins=[h1bs[k][:, :].opt()],
                    outs=[h1g[k * CHR:(k + 1) * CHR, :].opt()])

            # ================= LAYER 1 scatter =================
            ps_by_b = {}
            for s in range(SB):
                gt = sp.tile([P, 8 * G1], bf, tag="gt1")
                ix = sp.tile([P, 8], dt.int32, tag="ix1")
                nc.sync.dma_start(out=ix[:, :], in_=idx_d[s * P:(s + 1) * P, :])
                nc.gpsimd.indirect_dma_start(
                    out=gt[:, :], out_offset=None,
                    in_=h1g[:, :],
                    in_offset=bass.IndirectOffsetOnAxis(ap=ix[:, :], axis=0))
                dr = sp.tile([P, 8], bf, tag="dr1")
                nc.sync.dma_start(out=dr[:, :], in_=drl_d[s * P:(s + 1) * P, :])
                Ssb = sp.tile([P, 8 * P], bf, tag="S1", bufs=2)
                nc.vector.tensor_tensor(
                    out=Ssb[:, :].rearrange("p (a b) -> p a b", b=P),
                    in0=dr[:, :].unsqueeze(2).to_broadcast([P, 8, P]),
                    in1=iotaP[:, :].unsqueeze(1).to_broadcast([P, 8, P]),
                    op=OP.is_equal)
                for j in range(8):
                    g = s * 8 + j
                    b = int(b_of_block[g])
                    if first_of_b[g]:
                        ps_by_b[b] = psS.tile([P, G1], f32, tag="agg", name="agg1")
                    psb = ps_by_b[b]
                    is_last = bool(last_of_b[g])
                    nc.tensor.matmul(
                        psb[:, :], lhsT=Ssb[:, j * P:(j + 1) * P],
                        rhs=gt[:, j * G1:(j + 1) * G1],
                        start=bool(first_of_b[g]),
                        stop=is_last and not sched["g1b_nz"])
                    if not is_last:
                        continue
                    # ---------- finalize dst block b ----------
                    if sched["g1b_nz"]:
                        nc.tensor.matmul(
                            psb[:, :],
                            lhsT=sdeg_s[:1, b * P:(b + 1) * P],
                            rhs=g1b_s[:1, :], start=False, stop=True,
                            skip_group_check=True)
                    g1r = qp.tile([P, G1], bf, tag="g1r")
                    nc.scalar.activation(g1r[:, :], psb[:, :], AF.Relu,
                                         scale=dinv_s[:, b:b + 1])
                    del ps_by_b[b]
                    psm = psB.tile([P, G1], f32, tag="b")
                    nc.tensor.matmul(psm[:, :],
                                     lhsT=mT_s[:3, b * P:(b + 1) * P],
                                     rhs=W12_s[:3, :], start=True, stop=True)
                    g1t = qp.tile([P, G1], bf, tag="g1t")
                    nc.vector.tensor_mul(g1t[:, :], g1r[:, :], psm[:, :])
                    g1v = qp.tile([P, G1], bf, tag="g1v")
                    nc.vector.tensor_add(g1v[:, :], g1t[:, :],
                                         z_s[:, b * G1:(b + 1) * G1])
                    psm2 = psB.tile([P, G1], f32, tag="b")
                    nc.tensor.matmul(psm2[:, :],
                                     lhsT=mT_s[:3, b * P:(b + 1) * P],
                                     rhs=W2s_s[:3, :], start=True, stop=True)
                    r2 = qp.tile([P, G1], bf, tag="r2")
                    nc.vector.tensor_mul(r2[:, :], g1v[:, :], psm2[:, :])
                    if sched["bp2_nz"]:
                        psm3 = psB.tile([P, G1], f32, tag="b")
                        nc.tensor.matmul(psm3[:, :],
                                         lhsT=mT_s[:3, b * P:(b + 1) * P],
                                         rhs=bp2_s[:3, :], start=True,
                                         stop=True)
                        r2b = qp.tile([P, G1], bf, tag="r2b")
                        nc.vector.tensor_add(r2b[:, :], r2[:, :], psm3[:, :])
                        r2 = r2b
                    r2T = qp.tile([P, KG1 * P], bf, tag="r2T")
                    for f in range(KG1):
                        pst = psT.tile([P, P], bf, tag="t")
                        nc.tensor.transpose(pst[:, :],
                                            r2[:, f * P:(f + 1) * P],
                                            ident[:, :])
                        nc.vector.tensor_copy(r2T[:, f * P:(f + 1) * P],
                                              pst[:, :])
                    psh2 = psB.tile([P, G1], f32, tag="b")
                    for f in range(KG1):
                        nc.tensor.matmul(
                            psh2[:, :G2], lhsT=r2T[:, f * P:(f + 1) * P],
                            rhs=g2w_s[:, f * G2:(f + 1) * G2],
                            start=(f == 0), stop=(f == KG1 - 1))
                    h2p = qp.tile([P, G2], bf, tag="h2p")
                    nc.scalar.activation(h2p[:, :], psh2[:, :G2], AF.Copy,
                                         scale=dinv_s[:, b:b + 1])
                    kb2, rb2 = b // BPC, b % BPC
                    nc.sync.dma_start(
                        out=h2bs[kb2][rb2 * P:(rb2 + 1) * P, :], in_=h2p[:, :])
                    if rb2 == BPC - 1:
                        nc.gpsimd.collective_compute(
                            "AllGather", OP.bypass,
                            replica_groups=[list(range(cfg.NC))],
                            ins=[h2bs[kb2][:, :].opt()],
                            outs=[h2g[kb2 * CHR:(kb2 + 1) * CHR, :].opt()])

            # ================= LAYER 2 scatter =================
            SB2 = nblocks // 32
            ps_by_b2 = {}
            for s2 in range(SB2):
                gt2 = sp.tile([P, 32 * G2], bf, tag="gt2", bufs=2)
                ix2 = sp.tile([P, 32], dt.int32, tag="ix2")
                nc.sync.dma_start(
                    out=ix2[:, :].rearrange("p (a e) -> p a e", e=8),
                    in_=idx_d[s2 * 4 * P:(s2 + 1) * 4 * P, :]
                        .rearrange("(a p) e -> p a e", p=P))
                nc.gpsimd.indirect_dma_start(
                    out=gt2[:, :], out_offset=None,
                    in_=h2g[:, :],
                    in_offset=bass.IndirectOffsetOnAxis(ap=ix2[:, :], axis=0))
                dr2 = sp.tile([P, 32], bf, tag="dr2")
                nc.sync.dma_start(
                    out=dr2[:, :].rearrange("p (a e) -> p a e", e=8),
                    in_=drl_d[s2 * 4 * P:(s2 + 1) * 4 * P, :]
                        .rearrange("(a p) e -> p a e", p=P))
                S2 = sp.tile([P, 32 * P], bf, tag="S2", bufs=2)
                for half in range(2):
                    nc.vector.tensor_tensor(
                        out=S2[:, half * 16 * P:(half + 1) * 16 * P]
                            .rearrange("p (a b) -> p a b", b=P),
                        in0=dr2[:, half * 16:(half + 1) * 16]
                            .unsqueeze(2).to_broadcast([P, 16, P]),
                        in1=iotaP[:, :].unsqueeze(1).to_broadcast([P, 16, P]),
                        op=OP.is_equal)
                for j in range(32):
                    g = s2 * 32 + j
                    b = int(b_of_block[g])
                    if first_of_b[g]:
                        ps_by_b2[b] = psS.tile([P, G1], f32, tag="agg", name="agg2")
                    psb2 = ps_by_b2[b]
                    is_last = bool(last_of_b[g])
                    nc.tensor.matmul(
                        psb2[:, :G2], lhsT=S2[:, j * P:(j + 1) * P],
                        rhs=gt2[:, j * G2:(j + 1) * G2],
                        start=bool(first_of_b[g]),
                        stop=is_last and not sched["g2b_nz"])
                    if not is_last:
                        continue
                    if sched["g2b_nz"]:
                        nc.tensor.matmul(
                            psb2[:, :G2],
                            lhsT=sdeg_s[:1, b * P:(b + 1) * P],
                            rhs=g2b_s[:1, :], start=False, stop=True,
                            skip_group_check=True)
                    g2t = qp.tile([P, G2], bf, tag="g2t")
                    nc.scalar.activation(g2t[:, :], psb2[:, :G2], AF.Relu,
                                         scale=dinv_s[:, b:b + 1])
                    del ps_by_b2[b]
                    pstg = psT.tile([P, P], bf, tag="t")
                    nc.tensor.transpose(pstg[:G2, :], g2t[:, :], ident[:, :])
                    g2T = qp.tile([G2, P], bf, tag="g2T")
                    nc.vector.tensor_copy(g2T[:, :], pstg[:G2, :])
                    psf = psB.tile([P, G1], f32, tag="b")
                    nc.tensor.matmul(psf[:, :FOUT], lhsT=g2T[:, :],
                                     rhs=fcw_s[:, :], start=True,
                                     stop=not sched["fcb_nz"])
                    if sched["fcb_nz"]:
                        nc.tensor.matmul(psf[:, :FOUT], lhsT=ones1[:1, :],
                                         rhs=fcb_s[:1, :], start=False,
                                         stop=True, skip_group_check=True)
                    negmax = qp.tile([P, 1], f32, tag="negmax")
                    nc.vector.reduce_max(negmax[:, :], psf[:, :FOUT],
                                         axis=AX.X, negate=True)
                    esum = qp.tile([P, 1], f32, tag="esum")
                    etile = qp.tile([P, FOUT], f32, tag="etile")
                    nc.scalar.activation(etile[:, :], psf[:, :FOUT], AF.Exp,
                                         bias=negmax[:, :1],
                                         accum_out=esum[:, :1])
                    lns = qp.tile([P, 1], f32, tag="lns")
                    nc.scalar.activation(lns[:, :], esum[:, :], AF.Ln)
                    offs = qp.tile([P, 1], f32, tag="offs")
                    nc.vector.tensor_tensor(offs[:, :], negmax[:, :],
                                            lns[:, :], op=OP.subtract)
                    ot = qp.tile([P, FOUT], f32, tag="ot")
                    nc.vector.tensor_scalar(ot[:, :], psf[:, :FOUT],
                                            offs[:, :1], None, OP.add)
                    nc.sync.dma_start(out=out_d[b * P:(b + 1) * P, :],
                                      in_=ot[:, :])
    return nc


_LAST_EXEC_NS = None


def run(inputs, cfg, trace=False, debug=False):
    global _LAST_EXEC_NS
    in_maps, sched = host_prep(inputs, cfg)
    nc = build(cfg, sched, debug=debug)
    nc.finalize()
    from concourse import bass_utils
    res = bass_utils.run_bass_kernel_spmd(
        nc, in_maps, core_ids=list(range(cfg.NC)), trace=trace)
    _LAST_EXEC_NS = res.exec_time_ns
    outs = [np.asarray(res.results[c]["out"])[:cfg.NLOC_RAW]
            for c in range(cfg.NC)]
    return np.concatenate(outs, 0).astype(np.float32)


def kernel(**inputs):
    return run(inputs, _Cfg(**CFG_FULL))
